# revision 1
# baseline (speedup 1.0000x reference)
"""CRF loss (log-likelihood per sequence) on 8 Trainium2 NeuronCores.

Strategy
--------
Data-parallel over batch: each core gets 16 of the 128 sequences, the tiny
(K,) / (K,K) transition params are replicated.  Inside a core:

* Denominator (log-partition) runs the forward algorithm in LINEAR space:
      q_{s+1} = (E^T q_s) * f_{s+1},   f_s = exp(em_s + LN_C),  E = exp(trans)
  with a constant prescale LN_C baked into f so magnitudes stay in fp32/bf16
  exponent range, plus one exact sum-renormalisation per chain (the applied
  reciprocal is logged back, so this is exact).  All state is bf16, matmul
  accumulation is fp32 PSUM.
* The 1024-step scan is latency-bound (PE->DVE->PE sync per step), so the
  sequential depth is halved by running a forward chain (s=0..511) and a
  backward chain (t=1023..511) concurrently and meeting in the middle:
      log Z = log sum_k alpha_511[k] * beta_511[k]  (+ logged renorms)
* Numerator (gold path score) is exact fp32: indirect-DMA gathers of
  em[b,s,tgt[b,s]], trans[tgt[s],tgt[s+1]], start[tgt[0]], end[tgt[-1]]
  straight from HBM, reduced on-chip.  Runs fully overlapped with the chains.
* Emissions stream: HWDGE loads f32 (s,k)-major tiles, PE transposes 128x128
  blocks, ACT applies exp (with the LN_C bias) writing resident bf16
  k-major tiles (fT) consumed by both chains.  Chunks are produced
  both-ends-first so neither chain waits.

masks are all ones for this problem spec (fill: "ones"); asserted host-side.
"""

import sys

for _p in ("/opt/trn_rl_repo",):
    if _p not in sys.path:
        sys.path.insert(0, _p)

import numpy as np

import concourse.bass as bass
import concourse.bacc as bacc
import concourse.mybir as mybir
from concourse.tile import TileContext
from concourse.masks import make_identity

B, S, K = 128, 1024, 256
NCORES = 8
BL = B // NCORES          # 16 sequences per core
H = K // 128              # 2 partition-halves of the state vector
NCHUNK = S // 128         # 8 production chunks of 128 steps
LN_C = -6.045             # measured mean log-growth per step (randn emissions)
SMID = S // 2             # meet point: fwd owns s<=511, bwd owns s>=512

F32 = mybir.dt.float32
BF16 = mybir.dt.bfloat16
I32 = mybir.dt.int32
Exp = mybir.ActivationFunctionType.Exp
Ln = mybir.ActivationFunctionType.Ln
X = mybir.AxisListType.X
ADD = mybir.AluOpType.add
MULT = mybir.AluOpType.mult
SHL = mybir.AluOpType.logical_shift_left


DEBUG_OUTS = False


def build_nc() -> bass.Bass:
    nc = bacc.Bacc()
    em_d = nc.dram_tensor("emissions", [BL, S, K], F32, kind="ExternalInput")
    tg_d = nc.dram_tensor("targets32", [BL, 2 * S], I32, kind="ExternalInput")
    st_d = nc.dram_tensor("start_transitions", [K], F32, kind="ExternalInput")
    en_d = nc.dram_tensor("end_transitions", [K], F32, kind="ExternalInput")
    tr_d = nc.dram_tensor("transitions", [K, K], F32, kind="ExternalInput")
    out_d = nc.dram_tensor("out", [BL], F32, kind="ExternalOutput")
    dbg = None
    if DEBUG_OUTS:
        dbg = {
            "dbg_num": nc.dram_tensor("dbg_num", [BL], F32,
                                      kind="ExternalOutput"),
            "dbg_lnz": nc.dram_tensor("dbg_lnz", [BL], F32,
                                      kind="ExternalOutput"),
            "dbg_renlog": nc.dram_tensor("dbg_renlog", [BL], F32,
                                         kind="ExternalOutput"),
        }

    with TileContext(nc) as tc:
        _build(tc, nc, em_d, tg_d, st_d, en_d, tr_d, out_d, dbg)
    nc.finalize()
    return nc


def _build(tc, nc, em_d, tg_d, st_d, en_d, tr_d, out_d, dbg=None):
    import contextlib

    ctx = contextlib.ExitStack()
    const = ctx.enter_context(tc.tile_pool(name="const", bufs=1))
    natp = ctx.enter_context(tc.tile_pool(name="natp", bufs=4))
    qf_p = ctx.enter_context(tc.tile_pool(name="qf", bufs=3))
    qb_p = ctx.enter_context(tc.tile_pool(name="qb", bufs=3))
    workp = ctx.enter_context(tc.tile_pool(name="work", bufs=2))
    ppsum = ctx.enter_context(tc.tile_pool(name="ppsum", bufs=2, space="PSUM"))
    fpsum = ctx.enter_context(tc.tile_pool(name="fpsum", bufs=2, space="PSUM"))
    bpsum = ctx.enter_context(tc.tile_pool(name="bpsum", bufs=2, space="PSUM"))

    # ---------------- constants ----------------
    ident_f = const.tile([128, 128], F32, tag="ident_f")
    make_identity(nc, ident_f[:])
    ident_b = const.tile([128, 128], BF16, tag="ident_b")
    make_identity(nc, ident_b[:])
    ones_col_b = const.tile([128, 1], BF16, tag="ones_col_b")
    nc.gpsimd.memset(ones_col_b[:], 1.0)
    ones_col_f = const.tile([128, 1], F32, tag="ones_col_f")
    nc.gpsimd.memset(ones_col_f[:], 1.0)
    ones_row_b = const.tile([1, 128], BF16, tag="ones_row_b")
    nc.gpsimd.memset(ones_row_b[:], 1.0)
    one_f = const.tile([1, 1], F32, tag="one_f")
    nc.gpsimd.memset(one_f[:], 1.0)
    renlog = const.tile([1, BL], F32, tag="renlog")
    nc.gpsimd.memset(renlog[:], 0.0)
    bias_lnc = const.tile([128, 1], F32, tag="bias_lnc")
    nc.gpsimd.memset(bias_lnc[:], LN_C)
    bias_nlnc = const.tile([128, 1], F32, tag="bias_nlnc")
    nc.gpsimd.memset(bias_nlnc[:], -LN_C)
    # ACT instructions encode at most ONE sync wait; pre-absorb the Pool
    # (memset) dependency into ACT's vector clock so every later activation
    # only waits on its data producer.
    act_warm = const.tile([128, 1], F32, tag="act_warm")
    nc.scalar.copy(act_warm[:], bias_nlnc[:])

    # ---------------- transition matrices ----------------
    # E_sb[p, h, k'] = exp(trans)[h*128+p, k']  (bf16)
    tr_sb = const.tile([128, H, K], F32, tag="tr_sb")
    nc.sync.dma_start(tr_sb[:], tr_d[:].rearrange("(h p) k -> p h k", p=128))
    E_sb = const.tile([128, H, K], BF16, tag="E_sb")
    nc.scalar.activation(E_sb[:], tr_sb[:], Exp)
    # ET_sb[p, hc, ho*128+m] = E[ho*128+m, hc*128+p]   (transposed blocks)
    ET_sb = const.tile([128, H, K], BF16, tag="ET_sb")
    for hc in range(H):
        for ho in range(H):
            tp = ppsum.tile([128, 128], BF16, tag="pp")
            nc.tensor.transpose(
                tp[:], E_sb[:, ho, hc * 128:(hc + 1) * 128], ident_b[:]
            )
            nc.vector.tensor_copy(ET_sb[:, hc, ho * 128:(ho + 1) * 128], tp[:])

    # start/end vectors: (128, H) layout, k = h*128 + p
    st_sb = const.tile([128, H], F32, tag="st_sb")
    nc.sync.dma_start(st_sb[:], st_d[:].rearrange("(h p) -> p h", p=128))
    en_sb = const.tile([128, H], F32, tag="en_sb")
    nc.sync.dma_start(en_sb[:], en_d[:].rearrange("(h p) -> p h", p=128))
    S_exp = const.tile([128, H, 1], BF16, tag="S_exp")   # exp(start - LN_C)
    nc.scalar.activation(S_exp[:, :, 0], st_sb[:], Exp, bias=bias_nlnc[:])
    En_exp = const.tile([128, H, 1], BF16, tag="En_exp")  # exp(end)
    nc.scalar.activation(En_exp[:, :, 0], en_sb[:], Exp)

    # ---------------- targets + numerator gather indices ----------------
    t_sb = const.tile([16, 2 * S], I32, tag="t_sb")
    nc.sync.dma_start(t_sb[:], tg_d[:])
    t_low = t_sb[:].rearrange("p (s two) -> p s two", two=2)[:, :, 0]  # (16,S)

    # Indirect DMA gathers ONE contiguous run per partition-row, one index
    # per row.  So scalar gathers are laid out (128 rows, 1 elem) x 128
    # instructions, rows mapped p = sc*16 + b (sc = s//128 chunk, b = seq).
    # tgt2_raw[p, 2r] = targets[b, sc*128 + r] (int64 low words, stride 2)
    tgt2_raw = const.tile([128, 2 * 128], I32, tag="tgt2_raw")
    for sc in range(8):
        nc.sync.dma_start(tgt2_raw[sc * 16:(sc + 1) * 16, :],
                          tg_d[:, sc * 256:(sc + 1) * 256])
    t2 = tgt2_raw[:].rearrange("p (r two) -> p r two", two=2)[:, :, 0]  # (128,128)

    pidx = const.tile([128, 1], I32, tag="pidx")
    nc.gpsimd.iota(pidx[:], pattern=[[1, 1]], base=0, channel_multiplier=1)
    bpart = const.tile([128, 1], I32, tag="bpart")      # b = p % 16
    nc.vector.tensor_scalar(bpart[:], pidx[:], 15, None,
                            op0=mybir.AluOpType.bitwise_and)
    scpart = const.tile([128, 1], I32, tag="scpart")    # sc = p // 16
    nc.vector.tensor_scalar(scpart[:], pidx[:], 4, None,
                            op0=mybir.AluOpType.arith_shift_right)
    base = const.tile([128, 1], I32, tag="base")        # b*S*K + sc*128*K
    nc.vector.tensor_scalar(base[:], bpart[:], 18, None, op0=SHL)
    sctmp = const.tile([128, 1], I32, tag="sctmp")
    nc.vector.tensor_scalar(sctmp[:], scpart[:], 15, None, op0=SHL)
    nc.vector.tensor_tensor(base[:], base[:], sctmp[:], op=ADD)

    emt2 = const.tile([128, 128], I32, tag="emt2")      # flat em index
    nc.gpsimd.iota(emt2[:], pattern=[[K, 128]], base=0, channel_multiplier=0)
    nc.vector.tensor_tensor(emt2[:], emt2[:], base[:].to_broadcast([128, 128]),
                            op=ADD)
    nc.vector.tensor_tensor(emt2[:], emt2[:], t2, op=ADD)

    tr2 = const.tile([128, 128], I32, tag="tr2")        # t[s]*K + t[s+1]
    nc.vector.tensor_scalar(tr2[:, 0:127], t2[:, 0:127], 8, None, op0=SHL)
    nc.vector.tensor_tensor(tr2[:, 0:127], tr2[:, 0:127], t2[:, 1:128], op=ADD)
    # chunk-boundary transitions s = sc*128+127 -> sc*128+128 (7 per seq)
    bidx = const.tile([16, 8], I32, tag="bidx")
    tl3 = t_sb[:].rearrange("p (c r two) -> p c r two", two=2, r=128)
    nc.vector.tensor_scalar(bidx[:, 0:7], tl3[:, 0:7, 127, 0], 8, None, op0=SHL)
    nc.vector.tensor_tensor(bidx[:, 0:7], bidx[:, 0:7], tl3[:, 1:8, 0, 0],
                            op=ADD)
    # selection matrix Sel[p, m] = (p % 16 == m), for per-seq partition sums
    colio = const.tile([128, 16], I32, tag="colio")
    nc.gpsimd.iota(colio[:], pattern=[[1, 16]], base=0, channel_multiplier=0)
    colio_f = const.tile([128, 16], F32, tag="colio_f")
    nc.vector.tensor_copy(out=colio_f[:], in_=colio[:])
    bpart_f = const.tile([128, 1], F32, tag="bpart_f")
    nc.vector.tensor_copy(out=bpart_f[:], in_=bpart[:])
    self_sel = const.tile([128, 16], F32, tag="self_sel")
    nc.vector.tensor_scalar(self_sel[:], colio_f[:], bpart_f[:], None,
                            op0=mybir.AluOpType.is_equal)

    # ---------------- emissions stream -> fT chunks ----------------
    # fT[c][p, h, b, sl] = exp(em[b, c*128+sl, h*128+p] + LN_C)   (bf16)
    fT = [
        const.tile([128, H, BL, 128], BF16, tag=f"fT{c}", name=f"fT{c}")
        for c in range(NCHUNK)
    ]
    chunk_order = [0, 7, 1, 6, 2, 5, 3, 4]
    import contextlib as _ctxlib
    for sc in chunk_order:
        with (tc.high_priority(offset=-1_000_000) if sc not in (0, 7)
              else _ctxlib.nullcontext()):
            for bg in range(2):
                nat = natp.tile([128, 8, K], F32, tag="nat")
                src = em_d[bg * 8:(bg + 1) * 8, sc * 128:(sc + 1) * 128, :]
                nc.sync.dma_start(nat[:], src.rearrange("b s k -> s b k"))
                for h in range(H):
                    pp = ppsum.tile([128, 8, 128], F32, tag="pp")
                    for bl in range(8):
                        nc.tensor.transpose(
                            pp[:, bl, :], nat[:, bl, h * 128:(h + 1) * 128],
                            ident_f[:]
                        )
                    nc.scalar.activation(
                        fT[sc][:, h, bg * 8:(bg + 1) * 8, :], pp[:], Exp,
                        bias=bias_lnc[:]
                    )

    # ---------------- numerator gathers (exact fp32, from HBM) ----------
    stend_idx = const.tile([16, 2], I32, tag="stend_idx")
    nc.vector.tensor_copy(out=stend_idx[:, 0:1], in_=t_low[:, 0:1])
    nc.vector.tensor_copy(out=stend_idx[:, 1:2], in_=t_low[:, S - 1:S])
    emt_gv = const.tile([128, 128], F32, tag="emt_gv")
    trg_v = const.tile([128, 128], F32, tag="trg_v")
    btg = const.tile([16, 8], F32, tag="btg")
    with tc.high_priority(offset=-2_000_000):
        for r in range(128):
            nc.gpsimd.indirect_dma_start(
                out=emt_gv[:, r:r + 1], out_offset=None, in_=em_d[:],
                in_offset=bass.IndirectOffsetOnAxis(ap=emt2[:, r:r + 1],
                                                    axis=2),
            )
        for r in range(127):
            nc.gpsimd.indirect_dma_start(
                out=trg_v[:, r:r + 1], out_offset=None, in_=tr_d[:],
                in_offset=bass.IndirectOffsetOnAxis(ap=tr2[:, r:r + 1],
                                                    axis=1),
            )
        for j in range(7):
            nc.gpsimd.indirect_dma_start(
                out=btg[:, j:j + 1], out_offset=None, in_=tr_d[:],
                in_offset=bass.IndirectOffsetOnAxis(ap=bidx[:, j:j + 1],
                                                    axis=1),
            )
    st_g = const.tile([16, 1], F32, tag="st_g")
    nc.gpsimd.indirect_dma_start(
        out=st_g[:], out_offset=None,
        in_=st_d[:].rearrange("(a k) -> a k", a=1),
        in_offset=bass.IndirectOffsetOnAxis(ap=stend_idx[:, 0:1], axis=1),
    )
    en_g = const.tile([16, 1], F32, tag="en_g")
    nc.gpsimd.indirect_dma_start(
        out=en_g[:], out_offset=None,
        in_=en_d[:].rearrange("(a k) -> a k", a=1),
        in_offset=bass.IndirectOffsetOnAxis(ap=stend_idx[:, 1:2], axis=1),
    )

    # ---------------- chain helpers ----------------
    def flat(ap):  # (128, H, BL) -> (128, H*BL)
        return ap.rearrange("p h b -> p (h b)")

    def fwd_matmul(q_prev_ap):
        ps = fpsum.tile([128, H, BL], F32, tag="qp_f")
        for ho in range(H):
            for hi in range(H):
                nc.tensor.matmul(
                    ps[:, ho, :],
                    lhsT=E_sb[:, hi, ho * 128:(ho + 1) * 128],
                    rhs=q_prev_ap[:, hi, :],
                    start=(hi == 0), stop=(hi == H - 1),
                )
        return ps

    def bwd_matmul(u_ap):
        ps = bpsum.tile([128, H, BL], F32, tag="qp_b")
        for ho in range(H):
            for hc in range(H):
                nc.tensor.matmul(
                    ps[:, ho, :],
                    lhsT=ET_sb[:, hc, ho * 128:(ho + 1) * 128],
                    rhs=u_ap[:, hc, :],
                    start=(hc == 0), stop=(hc == H - 1),
                )
        return ps

    def renorm(q_ap, psum_pool, tag):
        """Divide q by its per-sequence sum (both halves), log the factor."""
        ps = psum_pool.tile([1, H * BL], F32, tag=tag)
        nc.tensor.matmul(ps[:], lhsT=ones_col_b[:], rhs=flat(q_ap),
                         start=True, stop=True)
        ps_sb = workp.tile([1, H * BL], F32, tag="ps_sb")
        nc.vector.tensor_copy(out=ps_sb[:], in_=ps[:])
        tot = workp.tile([1, BL], F32, tag="tot")
        nc.vector.tensor_tensor(tot[:], ps_sb[:, 0:BL], ps_sb[:, BL:2 * BL],
                                op=ADD)
        rinv = workp.tile([1, BL], F32, tag="rinv")
        nc.vector.reciprocal(rinv[:], tot[:])
        rinv2 = workp.tile([1, H, BL], BF16, tag="rinv2")
        for h in range(H):
            nc.vector.tensor_copy(out=rinv2[:, h, :], in_=rinv[:])
        pbc = psum_pool.tile([128, H * BL], F32, tag=tag)
        nc.tensor.matmul(pbc[:], lhsT=ones_row_b[:],
                         rhs=rinv2[:].rearrange("p h b -> p (h b)"),
                         start=True, stop=True)
        qn = (qf_p if psum_pool is fpsum else qb_p).tile(
            [128, H, BL], BF16, tag="q_f" if psum_pool is fpsum else "u_b")
        nc.vector.tensor_tensor(flat(qn[:]), flat(q_ap), pbc[:], op=MULT)
        # renlog -= ln(rinv) (i.e. += ln(tot_actual))
        lnr = workp.tile([1, BL], F32, tag="lnr")
        nc.scalar.activation(lnr[:], rinv2[:, 0, :], Ln)
        nc.vector.tensor_tensor(renlog[:], renlog[:], lnr[:], op=ADD)
        return qn

    def ft_at(s):
        return fT[s // 128][:, :, :, s % 128]

    # ---------------- chain initialisation ----------------
    # fwd: q_0 = exp(start - LN_C) * fT_0   ( = exp(start + em_0) )
    q_f = qf_p.tile([128, H, BL], BF16, tag="q_f")
    nc.vector.tensor_tensor(
        q_f[:], ft_at(0), S_exp[:].to_broadcast([128, H, BL]), op=MULT)
    # bwd: b_{S-1} = exp(end); first "u" multiply uses the broadcast directly
    b_prev_ap = None  # PSUM ap of b_{t+1}; None means use En_exp broadcast

    # ---------------- the two chains, interleaved ----------------
    RENORM_F = 256     # fwd renorm after this step
    RENORM_B = S - 2 - 256   # bwd renorm after this t
    for i in range(SMID):
        # ---- fwd step s = i+1 (fwd has 511 steps: s = 1..511) ----
        s = i + 1
        if s <= SMID - 1:
            ps = fwd_matmul(q_f[:])
            q_f = qf_p.tile([128, H, BL], BF16, tag="q_f")
            nc.vector.tensor_tensor(flat(q_f[:]), flat(ps[:]),
                                    ft_at(s).rearrange("p h b -> p (h b)"),
                                    op=MULT)
            if s == RENORM_F:
                q_f = renorm(q_f[:], fpsum, "qp_f")
        # ---- bwd step t = S-2-i  (t from 1022 down to 511) ----
        t = S - 2 - i
        u = qb_p.tile([128, H, BL], BF16, tag="u_b")
        if b_prev_ap is None:
            nc.vector.tensor_tensor(
                u[:], ft_at(t + 1), En_exp[:].to_broadcast([128, H, BL]),
                op=MULT)
        else:
            nc.vector.tensor_tensor(flat(u[:]), flat(b_prev_ap),
                                    ft_at(t + 1).rearrange("p h b -> p (h b)"),
                                    op=MULT)
        b_prev_ap = bwd_matmul(u[:])[:]
        if t == RENORM_B:
            ub = qb_p.tile([128, H, BL], BF16, tag="u_b")
            nc.vector.tensor_copy(out=flat(ub[:]), in_=flat(b_prev_ap))
            ub = renorm(ub[:], bpsum, "qp_b")
            b_prev_ap = ub[:]

    # after loop: q_f = alpha_511 (SBUF bf16), b_prev_ap = beta_511 (PSUM f32)
    # ---------------- meet in the middle ----------------
    dot = workp.tile([128, H, BL], F32, tag="dot")
    nc.vector.tensor_tensor(flat(dot[:]), flat(b_prev_ap), flat(q_f[:]),
                            op=MULT)
    pd = fpsum.tile([1, H * BL], F32, tag="qp_f")
    nc.tensor.matmul(pd[:], lhsT=ones_col_f[:], rhs=flat(dot[:]),
                     start=True, stop=True)
    pd_sb = workp.tile([1, H * BL], F32, tag="pd_sb")
    nc.vector.tensor_copy(out=pd_sb[:], in_=pd[:])
    zsum = workp.tile([1, BL], F32, tag="zsum")
    nc.vector.tensor_tensor(zsum[:], pd_sb[:, 0:BL], pd_sb[:, BL:2 * BL],
                            op=ADD)
    lnz = workp.tile([1, BL], F32, tag="lnz")
    nc.scalar.activation(lnz[:], zsum[:], Ln)

    den = workp.tile([1, BL], F32, tag="den")
    nc.vector.tensor_sub(den[:], lnz[:], renlog[:])
    nc.vector.tensor_scalar_add(den[:], den[:], -float(S - 1) * LN_C)

    # ---------------- numerator reductions ----------------
    em_red = workp.tile([128, 1], F32, tag="em_red")
    nc.vector.tensor_reduce(em_red[:], emt_gv[:], axis=X, op=ADD)
    tr_red = workp.tile([128, 1], F32, tag="tr_red")
    nc.vector.tensor_reduce(tr_red[:], trg_v[:, 0:127], axis=X, op=ADD)
    pnum = fpsum.tile([16, 1], F32, tag="qp_f")
    nc.tensor.matmul(pnum[:], lhsT=self_sel[:], rhs=em_red[:],
                     start=True, stop=False)
    nc.tensor.matmul(pnum[:], lhsT=self_sel[:], rhs=tr_red[:],
                     start=False, stop=True)
    bred = workp.tile([16, 1], F32, tag="bred")
    nc.vector.tensor_reduce(bred[:], btg[:, 0:7], axis=X, op=ADD)
    num = workp.tile([16, 1], F32, tag="num")
    nc.vector.tensor_tensor(num[:], pnum[:], bred[:], op=ADD)
    nc.vector.tensor_tensor(num[:], num[:], st_g[:], op=ADD)
    nc.vector.tensor_tensor(num[:], num[:], en_g[:], op=ADD)

    # ---------------- output ----------------
    pt = fpsum.tile([BL, 1], F32, tag="qp_f")
    nc.tensor.transpose(pt[:], den[:], one_f[:])
    llh = workp.tile([16, 1], F32, tag="llh")
    nc.vector.tensor_sub(llh[:], num[:], pt[:])
    nc.sync.dma_start(out_d[:].rearrange("(b one) -> b one", one=1), llh[:])
    if dbg is not None:
        nc.sync.dma_start(
            dbg["dbg_num"][:].rearrange("(b one) -> b one", one=1), num[:])
        nc.sync.dma_start(dbg["dbg_lnz"][:].rearrange("(one b) -> one b", one=1),
                          lnz[:])
        nc.sync.dma_start(
            dbg["dbg_renlog"][:].rearrange("(one b) -> one b", one=1),
            renlog[:])

    ctx.close()


# ======================================================================
# host wrapper
# ======================================================================
_NC_CACHE = None


def _get_nc():
    global _NC_CACHE
    if _NC_CACHE is None:
        _NC_CACHE = build_nc()
    return _NC_CACHE


def _make_in_maps(emissions, targets, start_transitions, end_transitions,
                  transitions):
    emissions = np.ascontiguousarray(np.asarray(emissions, dtype=np.float32))
    tg = np.ascontiguousarray(np.asarray(targets, dtype=np.int64))
    tg32 = tg.view(np.int32).reshape(B, 2 * S)
    st = np.ascontiguousarray(np.asarray(start_transitions, dtype=np.float32))
    en = np.ascontiguousarray(np.asarray(end_transitions, dtype=np.float32))
    tr = np.ascontiguousarray(np.asarray(transitions, dtype=np.float32))
    maps = []
    for i in range(NCORES):
        sl = slice(i * BL, (i + 1) * BL)
        maps.append({
            "emissions": np.ascontiguousarray(emissions[sl]),
            "targets32": np.ascontiguousarray(tg32[sl]),
            "start_transitions": st,
            "end_transitions": en,
            "transitions": tr,
        })
    return maps


def _run(in_maps, trace=False, **kw):
    from concourse.bass_utils import run_bass_kernel_spmd

    nc = _get_nc()
    return run_bass_kernel_spmd(nc, in_maps, core_ids=list(range(NCORES)),
                                trace=trace, **kw)


def kernel(emissions, targets, masks, start_transitions, end_transitions,
           transitions):
    assert np.asarray(masks).all(), "kernel assumes all-ones masks (spec fill)"
    in_maps = _make_in_maps(emissions, targets, start_transitions,
                            end_transitions, transitions)
    res = _run(in_maps)
    out = np.concatenate([np.asarray(res.results[i]["out"]).reshape(BL)
                          for i in range(NCORES)])
    return out.astype(np.float32)


# ======================================================================
# numpy reference (exact, fp64) for self-testing
# ======================================================================
def _ref_numpy(em, tgt, st, en, tr):
    Bq = em.shape[0]
    E = np.exp(tr.astype(np.float64))
    p = np.exp(st.astype(np.float64))[None, :] * np.exp(
        em[:, 0, :].astype(np.float64))
    acc = np.zeros(Bq)
    for s in range(1, S):
        f = np.exp(em[:, s, :].astype(np.float64))
        q = (p @ E) * f
        m = q.max(1)
        acc += np.log(m)
        p = q / m[:, None]
    den = acc + np.log((p * np.exp(en.astype(np.float64))[None, :]).sum(1))
    num = (st[tgt[:, 0]].astype(np.float64)
           + em[np.arange(Bq)[:, None], np.arange(S)[None, :], tgt].sum(1)
           + tr[tgt[:, :-1], tgt[:, 1:]].sum(1) + en[tgt[:, -1]])
    return num - den


def _selftest():
    """CoreSim validation on one core with synthetic data."""
    from concourse.bass_interp import CoreSim
    import time

    rng = np.random.default_rng(1)
    em = rng.standard_normal((BL, S, K)).astype(np.float32)
    tgt = rng.integers(0, K, (BL, S)).astype(np.int64)
    st = rng.uniform(-0.1, 0.1, K).astype(np.float32)
    en = rng.uniform(-0.1, 0.1, K).astype(np.float32)
    tr = rng.uniform(-0.1, 0.1, (K, K)).astype(np.float32)

    t0 = time.time()
    nc = build_nc()
    print(f"build+compile: {time.time()-t0:.1f}s", flush=True)

    sim = CoreSim(nc)
    m = _make_in_maps(
        np.broadcast_to(em, (B, S, K)).reshape(B, S, K) if False else
        np.concatenate([em] * NCORES, 0),
        np.concatenate([tgt] * NCORES, 0), st, en, tr)[0]
    for k, v in m.items():
        sim.tensor(k)[:] = v
    t0 = time.time()
    sim.simulate()
    print(f"sim wall: {time.time()-t0:.1f}s  sim.time: {sim.time} ns",
          flush=True)
    got = np.array(sim.tensor("out"))
    want = _ref_numpy(em, tgt, st, en, tr)
    rel = np.abs(got - want) / np.maximum(np.abs(want), 1e-6)
    print("got[:4] :", got[:4])
    print("want[:4]:", want[:4])
    print(f"max abs err {np.abs(got-want).max():.4f}  max rel {rel.max():.2e}")


if __name__ == "__main__":
    _selftest()

